# revision 1
# baseline (speedup 1.0000x reference)
# Causal self-attention (B=2, T=2048, D=1024, H=16, HD=64) with RoPE on 8 TRN2 cores.
#
# Sharding: data-parallel over batch (2 groups of 4 cores), tensor-parallel over
# heads within each group (4 heads per core). Each core computes, for its batch b
# and its 4 heads:
#   qkv^T projection (fp32r matmuls), RoPE on q/k, causal attention in a
#   transposed (S^T) layout with exp on the Scalar engine, AV with an augmented
#   ones-column producing the softmax denominator for free, and a row-sharded
#   out-projection producing a partial [D, T] output. The host sums the 4
#   partials per batch and transposes back.
#
# Everything on the PE runs in float32r (~13-bit mantissa, full speed at
# moving-dim >= 256). No max-subtraction in softmax: logits are ~N(0,1) here,
# exp never overflows.
import sys
import os

sys.path.insert(0, "/opt/trn_rl_repo")

import numpy as np

import concourse.bass as bass  # noqa: F401  (bass types used via bacc)
import concourse.mybir as mybir
from concourse import bacc
from concourse.tile import TileContext
from concourse.bass_utils import run_bass_kernel_spmd
from contextlib import ExitStack

F32 = mybir.dt.float32
F32R = mybir.dt.float32r
BF16 = mybir.dt.bfloat16
AF = mybir.ActivationFunctionType
ALU = mybir.AluOpType

B, T, D = 2, 2048, 1024
H, HD = 16, 64
NCORES = 8
GROUPS = NCORES // B          # cores per batch = 4
HPC = H // GROUPS             # heads per core = 4
NK = D // 128                 # contraction tiles for D
SCALE = HD ** -0.5

# hd interleave: new row 2j <- orig j, new row 2j+1 <- orig j+32 so the
# rotate-half partner of every row is its neighbour (swappable by a 32-lane
# stream shuffle).
PI = np.empty(HD, dtype=np.int64)
PI[0::2] = np.arange(32)
PI[1::2] = np.arange(32, 64)

SWAP_MASK = []
for _i in range(16):
    SWAP_MASK += [2 * _i + 1, 2 * _i]


def _sq_chunks(o, end=1024):
    """Chunks [pos, pos+cl) from o to end that never cross a 512-aligned PSUM
    bank boundary (a single matmul output must stay inside one bank)."""
    out = []
    pos = o
    while pos < end:
        nxt = min(end, (pos // 512 + 1) * 512)
        out.append((pos, nxt - pos))
        pos = nxt
    return out


def _build_program():
    nc = bacc.Bacc("TRN2", target_bir_lowering=False, debug=False,
                   num_devices=NCORES)
    d_xT = nc.dram_tensor("xT", [D, T], F32, kind="ExternalInput").ap()
    d_w = nc.dram_tensor("w_cat", [D, 6 * 128], F32, kind="ExternalInput").ap()
    d_wo = nc.dram_tensor("w_o", [2 * 128, D], F32, kind="ExternalInput").ap()
    d_cos = nc.dram_tensor("cos2", [128, T], F32, kind="ExternalInput").ap()
    d_sin = nc.dram_tensor("sin2", [128, T], F32, kind="ExternalInput").ap()
    d_id = nc.dram_tensor("ident", [128, 128], F32, kind="ExternalInput").ap()
    d_ones = nc.dram_tensor("ones16", [128, 16], F32, kind="ExternalInput").ap()
    d_out = nc.dram_tensor("outp", [D, T], BF16, kind="ExternalOutput").ap()
    dbg = bool(int(os.environ.get("KDEBUG", "0")))
    if dbg:
        d_dbg_q0 = nc.dram_tensor("dbg_q0", [128, T], F32, kind="ExternalOutput").ap()
        d_dbg_k0 = nc.dram_tensor("dbg_k0", [128, T], F32, kind="ExternalOutput").ap()
        d_dbg_va0 = nc.dram_tensor("dbg_va0", [128, 16 * 65], F32, kind="ExternalOutput").ap()
        d_dbg_o0 = nc.dram_tensor("dbg_o0", [128, T], F32, kind="ExternalOutput").ap()

    with TileContext(nc) as tc, nc.allow_low_precision(reason="f32r attention"):
        with ExitStack() as root:
            qkv_pool = root.enter_context(tc.tile_pool(name="qkv", bufs=1))
            va_pool = root.enter_context(tc.tile_pool(name="va", bufs=1))
            out_pool = root.enter_context(tc.tile_pool(name="outT", bufs=1))
            wop = root.enter_context(tc.tile_pool(name="wop", bufs=1))

            qT = [qkv_pool.tile([128, T], F32R, tag=f"q{p}", name=f"qT{p}")
                  for p in range(2)]
            kT = [qkv_pool.tile([128, T], F32R, tag=f"k{p}", name=f"kTt{p}")
                  for p in range(2)]
            va = [va_pool.tile([128, 16 * 65], F32R, tag=f"va{h}",
                               name=f"va{h}") for h in range(HPC)]
            oT = [out_pool.tile([128, T], F32R, tag=f"o{p}", name=f"oT{p}")
                  for p in range(2)]
            wo_sb = [wop.tile([128, D], F32R, tag=f"wo{p}", name=f"wo{p}")
                     for p in range(2)]

            # ---------------- Phase A: qkv^T projection + RoPE + v transpose
            with nc.named_scope("qkv"):
                with ExitStack() as sA:
                    tab = sA.enter_context(tc.tile_pool(name="tab", bufs=1))
                    xp = sA.enter_context(tc.tile_pool(name="xp", bufs=1))
                    wp = sA.enter_context(tc.tile_pool(name="wp", bufs=24))
                    tp = sA.enter_context(tc.tile_pool(name="ropetmp", bufs=1))
                    vtp = sA.enter_context(tc.tile_pool(name="vT", bufs=1))

                    cos2 = tab.tile([128, T], F32, tag="cos")
                    sin2 = tab.tile([128, T], F32, tag="sin")
                    ident = tab.tile([128, 128], F32R, tag="id")

                    x_sb = []
                    for kt in range(NK):
                        t_ = xp.tile([128, T], F32R, tag=f"x{kt}",
                                     name=f"xsb{kt}")
                        nc.sync.dma_start(
                            out=t_[:],
                            in_=d_xT[kt * 128:(kt + 1) * 128, :].bitcast(F32R))
                        x_sb.append(t_)

                    vT = [vtp.tile([128, T], F32R, tag=f"v{p}", name=f"vT{p}")
                          for p in range(2)]
                    qsh = tp.tile([128, T], F32, tag="qsh")
                    tcos = tp.tile([128, T], F32, tag="tcos")

                    def emit_proj(c, psum_pool, tag_prefix):
                        pc = []
                        for t in range(4):
                            pc.append(psum_pool.tile(
                                [128, 512], F32, tag=f"{tag_prefix}{t}",
                                name=f"pc{c}_{t}"))
                        for kt in range(NK):
                            w_t = wp.tile([128, 128], F32R, tag="w")
                            nc.scalar.dma_start(
                                out=w_t[:],
                                in_=d_w[kt * 128:(kt + 1) * 128,
                                        c * 128:(c + 1) * 128].bitcast(F32R))
                            for t in range(4):
                                nc.tensor.matmul(
                                    pc[t][:], w_t[:],
                                    x_sb[kt][:, t * 512:(t + 1) * 512],
                                    start=(kt == 0), stop=(kt == NK - 1))
                        return pc

                    def emit_rope(c, pc):
                        dst = qT[c - 2] if c < 4 else kT[c - 4]
                        for t in range(4):
                            sl = slice(t * 512, (t + 1) * 512)
                            nc.vector.stream_shuffle(qsh[:, sl], pc[t][:],
                                                     SWAP_MASK)
                            nc.vector.tensor_tensor(
                                out=tcos[:, sl], in0=pc[t][:],
                                in1=cos2[:, sl], op=ALU.mult)
                        nc.vector.tensor_tensor(out=qsh[:], in0=qsh[:],
                                                in1=sin2[:], op=ALU.mult)
                        nc.vector.tensor_tensor(out=dst[:], in0=qsh[:],
                                                in1=tcos[:], op=ALU.add)

                    # q0/k0 first (left PSUM stack) so pair-0 attention can
                    # begin while pair 1 is still projecting; v + transposes on
                    # the right stack.
                    psQK = tc.alloc_tile_pool(name="psQK", bufs=1,
                                              space="PSUM")
                    pc2 = emit_proj(2, psQK, "paq")
                    nc.scalar.dma_start(out=cos2[:], in_=d_cos[:])
                    nc.scalar.dma_start(out=sin2[:], in_=d_sin[:])
                    nc.scalar.dma_start(out=ident[:],
                                        in_=d_id[:].bitcast(F32R))
                    emit_rope(2, pc2)
                    pc = emit_proj(4, psQK, "paq")
                    emit_rope(4, pc)

                    psAv = tc.alloc_tile_pool(name="psAv", bufs=1,
                                              space="PSUM", side="right")
                    for c in range(2):
                        pc = emit_proj(c, psAv, "pav")
                        for t in range(4):
                            nc.scalar.copy(
                                vT[c][:, t * 512:(t + 1) * 512], pc[t][:])
                    psAv.release()
                    for h in range(HPC):
                        nc.sync.dma_start(out=va[h][:, 64:16 * 65:65],
                                          in_=d_ones[:].bitcast(F32R))
                    psT = tc.alloc_tile_pool(name="psT", bufs=4, space="PSUM",
                                             side="right")
                    for p in range(2):
                        for tt in range(16):
                            pt_ = psT.tile([128, 128], F32R, tag="pt",
                                           name=f"ptr{p}_{tt}")
                            nc.tensor.transpose(
                                pt_[:], vT[p][:, tt * 128:(tt + 1) * 128],
                                ident[:])
                            nc.scalar.copy(
                                va[2 * p][:, tt * 65:tt * 65 + 64],
                                pt_[:, 0:64])
                            nc.scalar.copy(
                                va[2 * p + 1][:, tt * 65:tt * 65 + 64],
                                pt_[:, 64:128])
                    psT.release()
                    # psS takes the right-side banks; pair-0 S/exp overlaps the
                    # pair-1 projection below.
                    psS = tc.alloc_tile_pool(name="psS", bufs=2, space="PSUM",
                                             side="right")
                    pc = emit_proj(3, psQK, "paq")
                    emit_rope(3, pc)
                    pc = emit_proj(5, psQK, "paq")
                    emit_rope(5, pc)
                    psQK.release()

            psV = tc.alloc_tile_pool(name="psV", bufs=2, space="PSUM")

            # ---------------- Phase B/C: causal attention, q-strips of 512
            with nc.named_scope("attn"):
                with ExitStack() as sB:
                    ptp = sB.enter_context(tc.tile_pool(name="ptp", bufs=10))
                    rp = sB.enter_context(tc.tile_pool(name="rp", bufs=6))

                    for si in range(4):
                        q0 = 512 * si
                        kb_max = 4 * (si + 1)
                        for p in range(2):
                            av = [psV.tile([65, 512], F32, tag=f"av{hl}",
                                           name=f"avps{si}_{p}_{hl}")
                                  for hl in range(2)]
                            for kb in range(kb_max):
                                o = max(0, 128 * kb - q0)
                                if o == 384:
                                    # widen to keep matmul moving-dim >= 256
                                    # (fp32r runs 4x slower below); the extra
                                    # columns are fully masked below.
                                    o = 256
                                # S^T for both heads into one [128, 1024]
                                # psum tile (head hl at cols hl*512+...)
                                sps = psS.tile([128, 1024], F32, tag="sps",
                                               name=f"sps{si}_{p}_{kb}")
                                for hl in range(2):
                                    hb = 64 * hl
                                    for pos, cl in _sq_chunks(o, 512):
                                        nc.tensor.matmul(
                                            sps[:, 512 * hl + pos:
                                                512 * hl + pos + cl],
                                            kT[p][hb:hb + 64,
                                                  kb * 128:(kb + 1) * 128],
                                            qT[p][hb:hb + 64,
                                                  q0 + pos:q0 + pos + cl],
                                            start=True, stop=True)
                                ptb = ptp.tile([128, 1024], F32R, tag="ptb",
                                               name=f"ptb{si}_{p}_{kb}")
                                L = 512 - o
                                sps3 = sps[:].rearrange(
                                    "a (h q) -> a h q", h=2)
                                ptb3 = ptb[:].rearrange(
                                    "a (h q) -> a h q", h=2)
                                nc.scalar.activation(
                                    ptb3[:, :, 0:L], sps3[:, :, o:512],
                                    AF.Exp, scale=SCALE)
                                # causal mask: keep col j of region iff
                                # (q0 + o + j) - (128*kb + part) >= 0
                                mbase = q0 + o - 128 * kb
                                mlen = min(128 - mbase, L)
                                if mlen > 0:
                                    for hl in range(2):
                                        nc.gpsimd.affine_select(
                                            ptb[:, 512 * hl:512 * hl + mlen],
                                            ptb[:, 512 * hl:512 * hl + mlen],
                                            pattern=[[1, mlen]],
                                            compare_op=ALU.is_ge, fill=0.0,
                                            base=mbase,
                                            channel_multiplier=-1)
                                for hl in range(2):
                                    h = 2 * p + hl
                                    for pos, cl in _sq_chunks(o, 512):
                                        nc.tensor.matmul(
                                            av[hl][:, pos:pos + cl],
                                            va[h][:, kb * 65:kb * 65 + 65],
                                            ptb[:, 512 * hl + pos - o:
                                                512 * hl + pos - o + cl],
                                            start=(kb == 0),
                                            stop=(kb == kb_max - 1),
                                            skip_group_check=True)
                            for hl in range(2):
                                r_sb = rp.tile([1, 512], F32, tag=f"r{hl}",
                                               name=f"rsb{si}_{p}_{hl}")
                                nc.vector.reciprocal(r_sb[:],
                                                     av[hl][64:65, :])
                                rb = rp.tile([64, 512], F32, tag=f"rb{hl}",
                                             name=f"rbb{si}_{p}_{hl}")
                                nc.gpsimd.partition_broadcast(rb[:], r_sb[:])
                                nc.vector.tensor_tensor(
                                    out=oT[p][64 * hl:64 * hl + 64,
                                              q0:q0 + 512],
                                    in0=av[hl][0:64, :], in1=rb[:],
                                    op=ALU.mult)

            psV.release()
            psS.release()

            if dbg:
                nc.sync.dma_start(out=d_dbg_q0[:], in_=qT[0][:].bitcast(F32))
                nc.sync.dma_start(out=d_dbg_k0[:], in_=kT[0][:].bitcast(F32))
                nc.sync.dma_start(out=d_dbg_va0[:], in_=va[0][:].bitcast(F32))
                nc.sync.dma_start(out=d_dbg_o0[:], in_=oT[0][:].bitcast(F32))

            # ---------------- Phase D: out-projection (row-sharded, partial)
            with nc.named_scope("oproj"):
                with ExitStack() as sD:
                    fop = sD.enter_context(tc.tile_pool(name="fop", bufs=6))
                    for p in range(2):
                        nc.scalar.dma_start(
                            out=wo_sb[p][:],
                            in_=d_wo[p * 128:(p + 1) * 128, :].bitcast(F32R))
                    psD = sD.enter_context(
                        tc.tile_pool(name="psD", bufs=1, space="PSUM"))
                    for t in range(4):
                        pD = [psD.tile([128, 512], F32, tag=f"pd{n}",
                                       name=f"pD{t}_{n}") for n in range(8)]
                        for p in range(2):
                            for n in range(8):
                                nc.tensor.matmul(
                                    pD[n][:],
                                    wo_sb[p][:, n * 128:(n + 1) * 128],
                                    oT[p][:, t * 512:(t + 1) * 512],
                                    start=(p == 0), stop=(p == 1))
                        for n in range(8):
                            fo = fop.tile([128, 512], BF16, tag="fo")
                            if n % 2 == 0:
                                nc.vector.tensor_copy(fo[:], pD[n][:])
                            else:
                                nc.scalar.copy(fo[:], pD[n][:])
                            nc.sync.dma_start(
                                out=d_out[n * 128:(n + 1) * 128,
                                          t * 512:(t + 1) * 512],
                                in_=fo[:])

    nc.compile()
    return nc


_NC_CACHE = None


def _get_program():
    global _NC_CACHE
    if _NC_CACHE is None:
        _NC_CACHE = _build_program()
    return _NC_CACHE


def _rope_tables():
    inv_freq = 1.0 / (10000.0 ** (np.arange(0, HD, 2, dtype=np.float32) / HD))
    freqs = np.outer(np.arange(T, dtype=np.float32), inv_freq)  # [T, 32]
    emb = np.concatenate([freqs, freqs], axis=-1)               # [T, 64]
    return np.cos(emb), np.sin(emb)


def _host_prep(x, w_qkv, w_out):
    cos, sin = _rope_tables()          # [T, 64] each, original hd order
    # permuted + transposed tables [64, T], duplicated for a 2-head pair tile
    cosP = np.ascontiguousarray(cos.T[PI, :])                   # [64, T]
    sinP = sin.T[PI, :].copy()                                  # [64, T]
    sinP[0::2, :] *= -1.0                                       # sign baked in
    cos2 = np.ascontiguousarray(np.vstack([cosP, cosP]), dtype=np.float32)
    sin2 = np.ascontiguousarray(np.vstack([sinP, sinP]), dtype=np.float32)
    ident = np.eye(128, dtype=np.float32)

    in_maps = []
    for core in range(NCORES):
        b = core // GROUPS
        h0 = (core % GROUPS) * HPC
        xT = np.ascontiguousarray(x[b].T)                       # [D, T]
        cols = []
        for p in range(2):                                      # v (no perm)
            for hh in range(2):
                h = h0 + 2 * p + hh
                cols.append(w_qkv[:, 2 * D + h * HD:2 * D + (h + 1) * HD])
        for kind in range(2):                                   # q, k
            for p in range(2):                                  # head pairs
                for hh in range(2):
                    h = h0 + 2 * p + hh
                    wcol = w_qkv[:, kind * D + h * HD:kind * D + (h + 1) * HD]
                    cols.append(wcol[:, PI])
        w_cat = np.ascontiguousarray(np.concatenate(cols, axis=1),
                                     dtype=np.float32)          # [D, 768]
        w_o = np.ascontiguousarray(
            w_out[h0 * HD:(h0 + HPC) * HD, :], dtype=np.float32)  # [256, D]
        in_maps.append({
            "xT": xT.astype(np.float32, copy=False),
            "w_cat": w_cat,
            "w_o": w_o,
            "cos2": cos2,
            "sin2": sin2,
            "ident": ident,
            "ones16": np.ones((128, 16), dtype=np.float32),
        })
    return in_maps


def kernel(x, w_qkv, w_out):
    x = np.asarray(x, dtype=np.float32)
    w_qkv = np.asarray(w_qkv, dtype=np.float32)
    w_out = np.asarray(w_out, dtype=np.float32)
    nc = _get_program()
    in_maps = _host_prep(x, w_qkv, w_out)
    trace = bool(int(os.environ.get("KBENCH_TRACE", "0")))
    res = run_bass_kernel_spmd(nc, in_maps, list(range(NCORES)), trace=trace)
    if trace and res.exec_time_ns is not None:
        print(f"HW exec time: {res.exec_time_ns} ns")
        if res.per_core_scope_times:
            for scope, cores in sorted(res.per_core_scope_times.items()):
                print(f"  scope {scope}: {cores}")
    out = np.zeros((B, T, D), dtype=np.float32)
    for core in range(NCORES):
        b = core // GROUPS
        out[b] += res.results[core]["outp"].T.astype(np.float32)
    return out



# revision 4
# speedup vs baseline: 1.2531x; 1.2531x over previous
# Causal self-attention (B=2, T=2048, D=1024, H=16, HD=64) with RoPE on 8 TRN2
# cores — bf16 pipeline.
#
# Sharding: data-parallel over batch (2 groups of 4 cores), tensor-parallel
# over heads within each group (4 heads per core, as 2 pairs of 2). Each core:
#   - streams xT in bf16 while projecting q(pair0) and v(pair0) per k-tile,
#   - k(pair0) strip-major with per-strip RoPE so attention starts early,
#   - v is computed directly in [keys, hd] layout (no PE transposes),
#   - causal attention in S^T layout: exp on ACT, one static triangle mask
#     multiplied on DVE for diagonal blocks, ones-column in the AV lhsT
#     produces softmax denominators for free; AV lags S/exp by one block,
#   - pair-1 projections / v / RoPE are interleaved into pair-0's attention
#     (ACT-bound), out-proj strips are interleaved into pair-1's attention,
#   - q-strips processed in descending si so the tail strip is the smallest.
# The host sums the per-core partial [D, T] outputs and transposes back.
import sys
import os

sys.path.insert(0, "/opt/trn_rl_repo")

import numpy as np

import concourse.bass as bass  # noqa: F401
import concourse.mybir as mybir
from concourse import bacc
from concourse.tile import TileContext
from concourse.bass_utils import run_bass_kernel_spmd
from contextlib import ExitStack

F32 = mybir.dt.float32
BF16 = mybir.dt.bfloat16
AF = mybir.ActivationFunctionType
ALU = mybir.AluOpType

B, T, D = 2, 2048, 1024
H, HD = 16, 64
NCORES = 8
GROUPS = NCORES // B          # cores per batch = 4
HPC = H // GROUPS             # heads per core = 4
NK = D // 128                 # contraction tiles for D = 8
SCALE = HD ** -0.5

# hd interleave: new row 2j <- orig j, new row 2j+1 <- orig j+32 so the
# rotate-half partner of every row is its neighbour (swappable by a 32-lane
# stream shuffle).
PI = np.empty(HD, dtype=np.int64)
PI[0::2] = np.arange(32)
PI[1::2] = np.arange(32, 64)

SWAP_MASK = []
for _i in range(16):
    SWAP_MASK += [2 * _i + 1, 2 * _i]

# w image chunk order (each chunk is one [D, 128] column block, swizzled so
# SBUF partition rows are contiguous in dram)
WC_Q0, WC_K0, WC_V0, WC_V1, WC_Q1, WC_K1 = range(6)


def _build_program():
    nc = bacc.Bacc("TRN2", target_bir_lowering=False, debug=False,
                   num_devices=NCORES)
    d_xT = nc.dram_tensor("xT", [D, T], BF16, kind="ExternalInput").ap()
    d_w = nc.dram_tensor("wimg", [128, 6 * NK * 128], BF16,
                         kind="ExternalInput").ap()
    d_wo = nc.dram_tensor("woimg", [128, 2 * D], BF16,
                          kind="ExternalInput").ap()
    d_cos = nc.dram_tensor("cos2", [128, T], BF16, kind="ExternalInput").ap()
    d_sin = nc.dram_tensor("sin2", [128, T], BF16, kind="ExternalInput").ap()
    d_mask = nc.dram_tensor("trimask", [128, 256], BF16,
                            kind="ExternalInput").ap()
    d_out = nc.dram_tensor("outp", [D, T], BF16, kind="ExternalOutput").ap()
    dbg = bool(int(os.environ.get("KDEBUG", "0")))
    if dbg:
        d_dbg_q0 = nc.dram_tensor("dbg_q0", [128, T], BF16,
                                  kind="ExternalOutput").ap()
        d_dbg_k0 = nc.dram_tensor("dbg_k0", [128, T], BF16,
                                  kind="ExternalOutput").ap()
        d_dbg_va0 = nc.dram_tensor("dbg_va0", [128, 2 * 16 * 65], BF16,
                                   kind="ExternalOutput").ap()
        d_dbg_o0 = nc.dram_tensor("dbg_o0", [128, T], BF16,
                                  kind="ExternalOutput").ap()

    with TileContext(nc) as tc, nc.allow_low_precision(reason="bf16 attn"):
        with ExitStack() as root:
            persist = root.enter_context(tc.tile_pool(name="persist", bufs=1))

            qT = [persist.tile([128, T], BF16, tag=f"q{p}", name=f"qT{p}")
                  for p in range(2)]
            kT = [persist.tile([128, T], BF16, tag=f"k{p}", name=f"kT{p}")
                  for p in range(2)]
            vap = [persist.tile([128, 2 * 16 * 65], BF16, tag=f"va{p}",
                                name=f"vap{p}") for p in range(2)]
            oT = [persist.tile([128, T], BF16, tag=f"o{p}", name=f"oT{p}")
                  for p in range(2)]
            x_sb = [persist.tile([128, T], BF16, tag=f"x{kt}",
                                 name=f"xsb{kt}") for kt in range(NK)]
            w_sb = [persist.tile([128, NK * 128], BF16, tag=f"w{c}",
                                 name=f"wsb{c}") for c in range(6)]
            wo_sb = [persist.tile([128, D], BF16, tag=f"wo{p}",
                                  name=f"wo{p}") for p in range(2)]
            cos2 = persist.tile([128, T], BF16, tag="cos")
            sin2 = persist.tile([128, T], BF16, tag="sin")
            trimask = persist.tile([128, 256], BF16, tag="m")

            ropep = root.enter_context(tc.tile_pool(name="ropep", bufs=1))
            qc = [ropep.tile([128, T], BF16, tag=f"qc{i}", name=f"qc{i}")
                  for i in range(2)]
            qsh = [ropep.tile([128, T], BF16, tag=f"qsh{i}", name=f"qsh{i}")
                   for i in range(2)]
            qco = [ropep.tile([128, T], BF16, tag=f"qco{i}", name=f"qco{i}")
                   for i in range(2)]

            ptbp = root.enter_context(tc.tile_pool(name="ptbp", bufs=3))
            rp = root.enter_context(tc.tile_pool(name="rp", bufs=2))
            fop = root.enter_context(tc.tile_pool(name="fop", bufs=4))

            # [128, hl, kb, 65] views of vap
            vap3 = [vap[p][:].rearrange("a (h k c) -> a h k c", h=2, k=16)
                    for p in range(2)]
            tri3 = trimask[:].rearrange("a (h c) -> a h c", h=2)

            # ---------------- DMA preamble (sync queue, ordered) ----------
            def wslice(c):
                return d_w[:, c * NK * 128:(c + 1) * NK * 128]

            nc.sync.dma_start(out=w_sb[WC_Q0][:], in_=wslice(WC_Q0))
            nc.sync.dma_start(out=w_sb[WC_V0][:], in_=wslice(WC_V0))
            for kt in range(NK):
                nc.sync.dma_start(out=x_sb[kt][:],
                                  in_=d_xT[kt * 128:(kt + 1) * 128, :])
            nc.sync.dma_start(out=w_sb[WC_K0][:], in_=wslice(WC_K0))
            nc.sync.dma_start(out=cos2[:], in_=d_cos[:])
            nc.sync.dma_start(out=sin2[:], in_=d_sin[:])
            nc.sync.dma_start(out=w_sb[WC_Q1][:], in_=wslice(WC_Q1))
            nc.sync.dma_start(out=w_sb[WC_K1][:], in_=wslice(WC_K1))
            nc.sync.dma_start(out=w_sb[WC_V1][:], in_=wslice(WC_V1))
            nc.sync.dma_start(out=wo_sb[0][:], in_=d_wo[:, 0:D])
            nc.sync.dma_start(out=wo_sb[1][:], in_=d_wo[:, D:2 * D])
            nc.sync.dma_start(out=trimask[:], in_=d_mask[:])

            for p in range(2):
                nc.vector.memset(vap3[p][:, :, :, 64:65], 1.0)

            # ---------------- rope helpers --------------------------------
            def rope_strip_ops(dst, buf, sl, src):
                """4 DVE ops turning src (bf16 or psum f32 [128, len(sl)])
                into roped dst[:, sl]."""
                nc.vector.tensor_tensor(out=qco[buf][:, sl], in0=src,
                                        in1=cos2[:, sl], op=ALU.mult)
                nc.vector.stream_shuffle(qsh[buf][:, sl], src, SWAP_MASK)
                nc.vector.tensor_tensor(out=qsh[buf][:, sl],
                                        in0=qsh[buf][:, sl],
                                        in1=sin2[:, sl], op=ALU.mult)
                nc.vector.tensor_tensor(out=dst[:, sl], in0=qsh[buf][:, sl],
                                        in1=qco[buf][:, sl], op=ALU.add)

            # ---------------- Phase A: x streaming + pair-0 projections ---
            psQ = tc.alloc_tile_pool(name="psQ", bufs=1, space="PSUM")
            psV = tc.alloc_tile_pool(name="psV", bufs=1, space="PSUM",
                                     side="right")
            pq = [psQ.tile([128, 512], F32, tag=f"pq{s}", name=f"pq{s}")
                  for s in range(4)]
            pv = [psV.tile([128, 512], F32, tag=f"pv{j}", name=f"pv{j}")
                  for j in range(4)]

            wq0 = w_sb[WC_Q0]
            wv0 = w_sb[WC_V0]
            for kt in range(NK):
                ks = slice(kt * 128, (kt + 1) * 128)
                for s in range(4):
                    nc.tensor.matmul(
                        pq[s][:], wq0[:, ks],
                        x_sb[kt][:, s * 512:(s + 1) * 512],
                        start=(kt == 0), stop=(kt == NK - 1))
                for kb in range(16):
                    nc.tensor.matmul(
                        pv[kb // 4][:, (kb % 4) * 128:(kb % 4 + 1) * 128],
                        x_sb[kt][:, kb * 128:(kb + 1) * 128],
                        wv0[:, ks],
                        start=(kt == 0), stop=(kt == NK - 1),
                        skip_group_check=True)

            # va copies for pair 0 (Pool), rope q0 (ACT copies + DVE, full-T)
            for kb in range(16):
                nc.gpsimd.tensor_copy(
                    vap3[0][:, :, kb, 0:64],
                    pv[kb // 4][:, (kb % 4) * 128:(kb % 4 + 1) * 128]
                    .rearrange("a (h c) -> a h c", h=2))
            for s in range(4):
                nc.scalar.copy(qc[0][:, s * 512:(s + 1) * 512], pq[s][:])
            rope_strip_ops(qT[0], 0, slice(0, T), qc[0][:])
            psQ.release()
            psV.release()

            # k0 proj strip-major with per-strip rope
            psK = tc.alloc_tile_pool(name="psK", bufs=2, space="PSUM")
            wk0 = w_sb[WC_K0]
            for s in range(4):
                pk = psK.tile([128, 512], F32, tag="pk", name=f"pk{s}")
                for kt in range(NK):
                    nc.tensor.matmul(
                        pk[:], wk0[:, kt * 128:(kt + 1) * 128],
                        x_sb[kt][:, s * 512:(s + 1) * 512],
                        start=(kt == 0), stop=(kt == NK - 1))
                sl = slice(s * 512, (s + 1) * 512)
                nc.scalar.copy(qc[1][:, sl], pk[:])
                rope_strip_ops(kT[0], 1, sl, qc[1][:, sl])
            psK.release()

            # ---------------- Phase B pools -------------------------------
            psS = tc.alloc_tile_pool(name="psS", bufs=2, space="PSUM",
                                     side="right")
            psA = tc.alloc_tile_pool(name="psA", bufs=1, space="PSUM")
            psBG = tc.alloc_tile_pool(name="psBG", bufs=1, space="PSUM")
            psD = tc.alloc_tile_pool(name="psD", bufs=1, space="PSUM")

            def bg_pair1():
                """q1/k1 proj + rope (DVE-direct from psum), then v1 + va
                copies. Yields between small PE chunks."""
                for wc, dstq, buf in ((WC_Q1, qT[1], 0), (WC_K1, kT[1], 1)):
                    w = w_sb[wc]
                    for s in range(4):
                        pk = psBG.tile([128, 512], F32, tag="bgk",
                                       name=f"bg{wc}_{s}")
                        for kt in range(0, NK, 2):
                            for k2 in (kt, kt + 1):
                                nc.tensor.matmul(
                                    pk[:], w[:, k2 * 128:(k2 + 1) * 128],
                                    x_sb[k2][:, s * 512:(s + 1) * 512],
                                    start=(k2 == 0), stop=(k2 == NK - 1))
                            yield
                        rope_strip_ops(dstq, buf,
                                       slice(s * 512, (s + 1) * 512), pk[:])
                        yield
                wv1 = w_sb[WC_V1]
                for kbq in range(4):
                    pv1 = psBG.tile([128, 512], F32, tag="bgk",
                                    name=f"bgv{kbq}")
                    for kj in range(4):
                        kb = 4 * kbq + kj
                        for kt in range(NK):
                            nc.tensor.matmul(
                                pv1[:, kj * 128:(kj + 1) * 128],
                                x_sb[kt][:, kb * 128:(kb + 1) * 128],
                                wv1[:, kt * 128:(kt + 1) * 128],
                                start=(kt == 0), stop=(kt == NK - 1),
                                skip_group_check=True)
                        yield
                    for kj in range(4):
                        kb = 4 * kbq + kj
                        nc.gpsimd.tensor_copy(
                            vap3[1][:, :, kb, 0:64],
                            pv1[:, kj * 128:(kj + 1) * 128]
                            .rearrange("a (h c) -> a h c", h=2))
                    yield

            def bg_oproj(si, pool):
                t0 = si * 512
                for n in range(8):
                    pD = pool.tile([128, 512], F32, tag="pd",
                                   name=f"pD{si}_{n}")
                    for p in range(2):
                        nc.tensor.matmul(
                            pD[:], wo_sb[p][:, n * 128:(n + 1) * 128],
                            oT[p][:, t0:t0 + 512],
                            start=(p == 0), stop=(p == 1))
                    yield
                    fo = fop.tile([128, 512], BF16, tag="fo")
                    if n % 2 == 0:
                        nc.vector.tensor_copy(fo[:], pD[:])
                    else:
                        nc.gpsimd.tensor_copy(fo[:], pD[:])
                    nc.scalar.dma_start(
                        out=d_out[n * 128:(n + 1) * 128, t0:t0 + 512],
                        in_=fo[:])
                    yield

            def drain(gen, count=10 ** 9):
                if gen is None:
                    return
                for _ in range(count):
                    try:
                        next(gen)
                    except StopIteration:
                        return

            def attn_strip(si, p, bg=None, bg_per_kb=2):
                """Attention for q-strip si, pair p; AV lags one block."""
                q0 = 512 * si
                kb_max = 4 * (si + 1)
                av = [psA.tile([65, 512], F32, tag=f"av{hl}",
                               name=f"av{si}_{p}_{hl}") for hl in range(2)]

                def emit_av(st):
                    kb, ptb, o, L = st
                    for hl in range(2):
                        nc.tensor.matmul(
                            av[hl][:, o:512],
                            vap3[p][:, hl, kb, :],
                            ptb[:, 512 * hl:512 * hl + L],
                            start=(kb == 0), stop=(kb == kb_max - 1),
                            skip_group_check=True)

                prev = None
                for kb in range(kb_max):
                    o = max(0, 128 * kb - q0)
                    L = 512 - o
                    sps = psS.tile([128, 1024], F32, tag="sps",
                                   name=f"sps{si}_{p}_{kb}")
                    for hl in range(2):
                        hb = 64 * hl
                        nc.tensor.matmul(
                            sps[:, 512 * hl + o:512 * hl + 512],
                            kT[p][hb:hb + 64, kb * 128:(kb + 1) * 128],
                            qT[p][hb:hb + 64, q0 + o:q0 + 512],
                            start=True, stop=True)
                    ptb = ptbp.tile([128, 1024], BF16, tag="ptb",
                                    name=f"ptb{si}_{p}_{kb}")
                    sps3 = sps[:].rearrange("a (h q) -> a h q", h=2)
                    ptb3 = ptb[:].rearrange("a (h q) -> a h q", h=2)
                    nc.scalar.activation(ptb3[:, :, 0:L], sps3[:, :, o:512],
                                         AF.Exp, scale=SCALE)
                    if kb >= 4 * si:
                        nc.vector.tensor_tensor(
                            out=ptb3[:, :, 0:128], in0=ptb3[:, :, 0:128],
                            in1=tri3, op=ALU.mult)
                    if prev is not None:
                        emit_av(prev)
                    prev = (kb, ptb, o, L)
                    if bg is not None:
                        drain(bg, count=bg_per_kb)
                emit_av(prev)
                # normalize -> oT strip
                for hl in range(2):
                    r_sb = rp.tile([1, 512], F32, tag=f"r{hl}",
                                   name=f"rsb{si}_{p}_{hl}")
                    nc.vector.reciprocal(r_sb[:], av[hl][64:65, :])
                    rb = rp.tile([64, 512], F32, tag=f"rb{hl}",
                                 name=f"rbb{si}_{p}_{hl}")
                    nc.gpsimd.partition_broadcast(rb[:], r_sb[:])
                    nc.vector.tensor_tensor(
                        out=oT[p][64 * hl:64 * hl + 64, q0:q0 + 512],
                        in0=av[hl][0:64, :], in1=rb[:], op=ALU.mult)

            # ---------------- Phase B: attention --------------------------
            bg1 = bg_pair1()
            for si in (3, 2, 1, 0):
                attn_strip(si, 0, bg=bg1, bg_per_kb=2)
            drain(bg1)

            obg = None
            for si in (3, 2, 1):
                attn_strip(si, 1, bg=obg, bg_per_kb=2)
                drain(obg)
                obg = bg_oproj(si, psD)
            attn_strip(0, 1, bg=obg, bg_per_kb=2)
            drain(obg)

            psD.release()
            psBG.release()
            psA.release()
            psS.release()

            # tail: out-projection of the last (smallest) strip with all
            # banks available
            psD2 = tc.alloc_tile_pool(name="psD2", bufs=4, space="PSUM")
            drain(bg_oproj(0, psD2))
            psD2.release()

            if dbg:
                nc.sync.dma_start(out=d_dbg_q0[:], in_=qT[0][:])
                nc.sync.dma_start(out=d_dbg_k0[:], in_=kT[0][:])
                nc.sync.dma_start(out=d_dbg_va0[:], in_=vap[0][:])
                nc.sync.dma_start(out=d_dbg_o0[:], in_=oT[0][:])

    nc.compile()
    return nc


_NC_CACHE = None


def _get_program():
    global _NC_CACHE
    if _NC_CACHE is None:
        _NC_CACHE = _build_program()
    return _NC_CACHE


def _rope_tables():
    inv_freq = 1.0 / (10000.0 ** (np.arange(0, HD, 2, dtype=np.float32) / HD))
    freqs = np.outer(np.arange(T, dtype=np.float32), inv_freq)  # [T, 32]
    emb = np.concatenate([freqs, freqs], axis=-1)               # [T, 64]
    return np.cos(emb), np.sin(emb)


def _to_bf16(a):
    import ml_dtypes
    return np.asarray(a, dtype=np.float32).astype(ml_dtypes.bfloat16)


def _swizzle_w(wcol):
    """[D, 128] column block -> SBUF image [128, NK*128] with
    img[p, kt*128 + j] = wcol[kt*128 + p, j]."""
    w3 = wcol.reshape(NK, 128, 128)          # [kt, p, j]
    return np.ascontiguousarray(w3.transpose(1, 0, 2).reshape(128, NK * 128))


def _host_prep(x, w_qkv, w_out):
    cos, sin = _rope_tables()
    cosP = np.ascontiguousarray(cos.T[PI, :])                   # [64, T]
    sinP = sin.T[PI, :].copy()
    sinP[0::2, :] *= -1.0                                       # sign baked in
    cos2 = _to_bf16(np.vstack([cosP, cosP]))
    sin2 = _to_bf16(np.vstack([sinP, sinP]))
    tri = np.triu(np.ones((128, 128), dtype=np.float32))        # keep j >= i
    trimask = _to_bf16(np.ascontiguousarray(
        np.concatenate([tri, tri], axis=1)))                    # [128, 256]

    in_maps = []
    for core in range(NCORES):
        b = core // GROUPS
        h0 = (core % GROUPS) * HPC
        xT = _to_bf16(np.ascontiguousarray(x[b].T))             # [D, T]

        def wcolq(kind, pair):                                  # permuted
            cols = []
            for hh in range(2):
                h = h0 + 2 * pair + hh
                wcol = w_qkv[:, kind * D + h * HD:kind * D + (h + 1) * HD]
                cols.append(wcol[:, PI])
            return np.concatenate(cols, axis=1)                 # [D, 128]

        def wcolv(pair):
            cols = []
            for hh in range(2):
                h = h0 + 2 * pair + hh
                cols.append(w_qkv[:, 2 * D + h * HD:2 * D + (h + 1) * HD])
            return np.concatenate(cols, axis=1)

        order = [wcolq(0, 0), wcolq(1, 0), wcolv(0), wcolv(1),
                 wcolq(0, 1), wcolq(1, 1)]
        wimg = np.concatenate([_swizzle_w(c) for c in order], axis=1)
        wimg = np.ascontiguousarray(_to_bf16(wimg))             # [128, 6144]

        wo_rows = w_out[h0 * HD:(h0 + HPC) * HD, :]             # [256, D]
        woimg = np.ascontiguousarray(_to_bf16(
            np.concatenate([wo_rows[0:128, :], wo_rows[128:256, :]],
                           axis=1)))                            # [128, 2D]

        in_maps.append({
            "xT": xT,
            "wimg": wimg,
            "woimg": woimg,
            "cos2": cos2,
            "sin2": sin2,
            "trimask": trimask,
        })
    return in_maps


def kernel(x, w_qkv, w_out):
    x = np.asarray(x, dtype=np.float32)
    w_qkv = np.asarray(w_qkv, dtype=np.float32)
    w_out = np.asarray(w_out, dtype=np.float32)
    nc = _get_program()
    in_maps = _host_prep(x, w_qkv, w_out)
    trace = bool(int(os.environ.get("KBENCH_TRACE", "0")))
    res = run_bass_kernel_spmd(nc, in_maps, list(range(NCORES)), trace=trace)
    if trace and res.exec_time_ns is not None:
        print(f"HW exec time: {res.exec_time_ns} ns")
    out = np.zeros((B, T, D), dtype=np.float32)
    for core in range(NCORES):
        b = core // GROUPS
        out[b] += res.results[core]["outp"].T.astype(np.float32)
    return out


# revision 8
# speedup vs baseline: 1.3326x; 1.0634x over previous
# Causal self-attention (B=2, T=2048, D=1024, H=16, HD=64) with RoPE on 8 TRN2
# cores — bf16 pipeline.
#
# Sharding: data-parallel over batch (2 groups of 4 cores), tensor-parallel
# over heads within each group (4 heads per core, as 2 pairs of 2). Each core:
#   - streams xT in bf16 while projecting q(pair0) and v(pair0) per k-tile,
#   - k(pair0) strip-major with per-strip RoPE so attention starts early,
#   - v is computed directly in [keys, hd] layout (no PE transposes),
#   - causal attention in S^T layout: exp on ACT, one static triangle mask
#     multiplied on DVE for diagonal blocks, ones-column in the AV lhsT
#     produces softmax denominators for free; AV lags S/exp by one block,
#   - pair-1 projections / v / RoPE are interleaved into pair-0's attention
#     (ACT-bound), out-proj strips are interleaved into pair-1's attention,
#   - q-strips processed in descending si so the tail strip is the smallest.
# The host sums the per-core partial [D, T] outputs and transposes back.
import sys
import os

sys.path.insert(0, "/opt/trn_rl_repo")

import numpy as np

import concourse.bass as bass  # noqa: F401
import concourse.mybir as mybir
from concourse import bacc
from concourse.tile import TileContext
from concourse.bass_utils import run_bass_kernel_spmd
from contextlib import ExitStack

F32 = mybir.dt.float32
BF16 = mybir.dt.bfloat16
AF = mybir.ActivationFunctionType
ALU = mybir.AluOpType

B, T, D = 2, 2048, 1024
H, HD = 16, 64
NCORES = 8
GROUPS = NCORES // B          # cores per batch = 4
HPC = H // GROUPS             # heads per core = 4
NK = D // 128                 # contraction tiles for D = 8
SCALE = HD ** -0.5

# hd interleave: new row 2j <- orig j, new row 2j+1 <- orig j+32 so the
# rotate-half partner of every row is its neighbour (swappable by a 32-lane
# stream shuffle).
PI = np.empty(HD, dtype=np.int64)
PI[0::2] = np.arange(32)
PI[1::2] = np.arange(32, 64)

SWAP_MASK = []
for _i in range(16):
    SWAP_MASK += [2 * _i + 1, 2 * _i]

# w image chunk order (each chunk is one [D, 128] column block, swizzled so
# SBUF partition rows are contiguous in dram)
WC_Q0, WC_K0, WC_V0, WC_V1, WC_Q1, WC_K1 = range(6)


def _build_program():
    nc = bacc.Bacc("TRN2", target_bir_lowering=False, debug=False,
                   num_devices=NCORES)
    d_xT = nc.dram_tensor("xT", [D, T], BF16, kind="ExternalInput").ap()
    d_w = nc.dram_tensor("wimg", [128, 6 * NK * 128], BF16,
                         kind="ExternalInput").ap()
    d_wo = nc.dram_tensor("woimg", [128, 2 * D], BF16,
                          kind="ExternalInput").ap()
    d_cos = nc.dram_tensor("cos2", [128, T], BF16, kind="ExternalInput").ap()
    d_sin = nc.dram_tensor("sin2", [128, T], BF16, kind="ExternalInput").ap()
    d_mask = nc.dram_tensor("trimask", [128, 256], BF16,
                            kind="ExternalInput").ap()
    d_out = nc.dram_tensor("outp", [D, T], BF16, kind="ExternalOutput").ap()
    dbg = bool(int(os.environ.get("KDEBUG", "0")))
    if dbg:
        d_dbg_q0 = nc.dram_tensor("dbg_q0", [128, T], BF16,
                                  kind="ExternalOutput").ap()
        d_dbg_k0 = nc.dram_tensor("dbg_k0", [128, T], BF16,
                                  kind="ExternalOutput").ap()
        d_dbg_va0 = nc.dram_tensor("dbg_va0", [128, 2 * 16 * 65], BF16,
                                   kind="ExternalOutput").ap()
        d_dbg_o0 = nc.dram_tensor("dbg_o0", [128, T], BF16,
                                  kind="ExternalOutput").ap()

    with TileContext(nc) as tc, nc.allow_low_precision(reason="bf16 attn"):
        with ExitStack() as root:
            persist = root.enter_context(tc.tile_pool(name="persist", bufs=1))

            qT = [persist.tile([128, T], BF16, tag=f"q{p}", name=f"qT{p}")
                  for p in range(2)]
            kT = [persist.tile([128, T], BF16, tag=f"k{p}", name=f"kT{p}")
                  for p in range(2)]
            vap = [persist.tile([128, 2 * 16 * 65], BF16, tag=f"va{p}",
                                name=f"vap{p}") for p in range(2)]
            oT = [persist.tile([128, T], BF16, tag=f"o{p}", name=f"oT{p}")
                  for p in range(2)]
            x_sb = [persist.tile([128, T], BF16, tag=f"x{kt}",
                                 name=f"xsb{kt}") for kt in range(NK)]
            w_sb = [persist.tile([128, NK * 128], BF16, tag=f"w{c}",
                                 name=f"wsb{c}") for c in range(6)]
            wo_sb = [persist.tile([128, D], BF16, tag=f"wo{p}",
                                  name=f"wo{p}") for p in range(2)]
            cos2 = persist.tile([128, T], BF16, tag="cos")
            sin2 = persist.tile([128, T], BF16, tag="sin")
            trimask = persist.tile([128, 256], BF16, tag="m")

            ropep = root.enter_context(tc.tile_pool(name="ropep", bufs=1))
            qc = [ropep.tile([128, T], BF16, tag=f"qc{i}", name=f"qc{i}")
                  for i in range(2)]
            qsh = [ropep.tile([128, T], BF16, tag=f"qsh{i}", name=f"qsh{i}")
                   for i in range(2)]
            qco = [ropep.tile([128, T], BF16, tag=f"qco{i}", name=f"qco{i}")
                   for i in range(2)]

            ptbp = root.enter_context(tc.tile_pool(name="ptbp", bufs=3))
            rp = root.enter_context(tc.tile_pool(name="rp", bufs=2))
            fop = root.enter_context(tc.tile_pool(name="fop", bufs=4))

            # [128, hl, kb, 65] views of vap
            vap3 = [vap[p][:].rearrange("a (h k c) -> a h k c", h=2, k=16)
                    for p in range(2)]
            tri3 = trimask[:].rearrange("a (h c) -> a h c", h=2)

            # ---------------- DMA preamble (sync queue, ordered) ----------
            def wslice(c):
                return d_w[:, c * NK * 128:(c + 1) * NK * 128]

            nc.sync.dma_start(out=w_sb[WC_Q0][:], in_=wslice(WC_Q0))
            nc.sync.dma_start(out=w_sb[WC_V0][:], in_=wslice(WC_V0))
            for kt in range(NK):
                nc.sync.dma_start(out=x_sb[kt][:],
                                  in_=d_xT[kt * 128:(kt + 1) * 128, :])
            nc.sync.dma_start(out=w_sb[WC_K0][:], in_=wslice(WC_K0))
            nc.sync.dma_start(out=cos2[:], in_=d_cos[:])
            nc.sync.dma_start(out=sin2[:], in_=d_sin[:])
            nc.sync.dma_start(out=w_sb[WC_Q1][:], in_=wslice(WC_Q1))
            nc.sync.dma_start(out=w_sb[WC_K1][:], in_=wslice(WC_K1))
            nc.sync.dma_start(out=w_sb[WC_V1][:], in_=wslice(WC_V1))
            nc.sync.dma_start(out=wo_sb[0][:], in_=d_wo[:, 0:D])
            nc.sync.dma_start(out=wo_sb[1][:], in_=d_wo[:, D:2 * D])
            nc.sync.dma_start(out=trimask[:], in_=d_mask[:])

            for p in range(2):
                nc.vector.memset(vap3[p][:, :, :, 64:65], 1.0)

            # ---------------- rope helpers --------------------------------
            def rope_strip_ops(dst, buf, sl, src):
                """4 DVE ops turning src (bf16 or psum f32 [128, len(sl)])
                into roped dst[:, sl]."""
                nc.vector.tensor_tensor(out=qco[buf][:, sl], in0=src,
                                        in1=cos2[:, sl], op=ALU.mult)
                nc.vector.stream_shuffle(qsh[buf][:, sl], src, SWAP_MASK)
                nc.vector.tensor_tensor(out=qsh[buf][:, sl],
                                        in0=qsh[buf][:, sl],
                                        in1=sin2[:, sl], op=ALU.mult)
                nc.vector.tensor_tensor(out=dst[:, sl], in0=qsh[buf][:, sl],
                                        in1=qco[buf][:, sl], op=ALU.add)

            # ---------------- Phase A: x streaming + pair-0 projections ---
            psQ = tc.alloc_tile_pool(name="psQ", bufs=1, space="PSUM")
            psV = tc.alloc_tile_pool(name="psV", bufs=1, space="PSUM",
                                     side="right")
            pq = [psQ.tile([128, 512], F32, tag=f"pq{s}", name=f"pq{s}")
                  for s in range(4)]
            pv = [psV.tile([128, 512], F32, tag=f"pv{j}", name=f"pv{j}")
                  for j in range(4)]

            wq0 = w_sb[WC_Q0]
            wv0 = w_sb[WC_V0]
            for kt in range(NK):
                ks = slice(kt * 128, (kt + 1) * 128)
                for s in range(4):
                    nc.tensor.matmul(
                        pq[s][:], wq0[:, ks],
                        x_sb[kt][:, s * 512:(s + 1) * 512],
                        start=(kt == 0), stop=(kt == NK - 1))
                for kb in range(16):
                    nc.tensor.matmul(
                        pv[kb // 4][:, (kb % 4) * 128:(kb % 4 + 1) * 128],
                        x_sb[kt][:, kb * 128:(kb + 1) * 128],
                        wv0[:, ks],
                        start=(kt == 0), stop=(kt == NK - 1),
                        skip_group_check=True)

            # va copies for pair 0 (Pool); rope q0 per strip (ACT copy + DVE),
            # strip 3 first since attention processes si descending; k0 proj
            # strip-major (ascending — attention consumes k strips from 0)
            # interleaved between q0 rope strips.
            wk0 = w_sb[WC_K0]
            for kb in range(4):
                nc.gpsimd.tensor_copy(
                    vap3[0][:, :, kb, 0:64],
                    pv[kb // 4][:, (kb % 4) * 128:(kb % 4 + 1) * 128]
                    .rearrange("a (h c) -> a h c", h=2))

            def emit_k0_strip(s, qs):
                # reuse the q-strip psum bank freed by the qc copy above
                pk = psQ.tile([128, 512], F32, tag=f"pq{qs}",
                              name=f"pk{s}")
                for kt in range(NK):
                    nc.tensor.matmul(
                        pk[:], wk0[:, kt * 128:(kt + 1) * 128],
                        x_sb[kt][:, s * 512:(s + 1) * 512],
                        start=(kt == 0), stop=(kt == NK - 1))
                sl = slice(s * 512, (s + 1) * 512)
                nc.scalar.copy(qc[1][:, sl], pk[:])
                rope_strip_ops(kT[0], 1, sl, qc[1][:, sl])

            for i, (qs, ks) in enumerate(((3, 0), (2, 1), (1, 2), (0, 3))):
                sl = slice(qs * 512, (qs + 1) * 512)
                nc.scalar.copy(qc[0][:, sl], pq[qs][:])
                rope_strip_ops(qT[0], 0, sl, qc[0][:, sl])
                emit_k0_strip(ks, qs)
                for kb in range(4 * (i + 1), min(16, 4 * (i + 2))):
                    nc.gpsimd.tensor_copy(
                        vap3[0][:, :, kb, 0:64],
                        pv[kb // 4][:, (kb % 4) * 128:(kb % 4 + 1) * 128]
                        .rearrange("a (h c) -> a h c", h=2))
            psQ.release()
            psV.release()

            # ---------------- Phase B pools -------------------------------
            psS = tc.alloc_tile_pool(name="psS", bufs=2, space="PSUM",
                                     side="right")
            psA = tc.alloc_tile_pool(name="psA", bufs=1, space="PSUM")
            psBG = tc.alloc_tile_pool(name="psBG", bufs=1, space="PSUM")
            psD = tc.alloc_tile_pool(name="psD", bufs=1, space="PSUM")

            def bg_pair1():
                """q1/k1 proj + rope (DVE-direct from psum), then v1 + va
                copies. Yields between small PE chunks."""
                for wc, dstq, buf in ((WC_Q1, qT[1], 0), (WC_K1, kT[1], 1)):
                    w = w_sb[wc]
                    for s in range(4):
                        pk = psBG.tile([128, 512], F32, tag="bgk",
                                       name=f"bg{wc}_{s}")
                        for kt in range(0, NK, 2):
                            for k2 in (kt, kt + 1):
                                nc.tensor.matmul(
                                    pk[:], w[:, k2 * 128:(k2 + 1) * 128],
                                    x_sb[k2][:, s * 512:(s + 1) * 512],
                                    start=(k2 == 0), stop=(k2 == NK - 1))
                            yield
                        rope_strip_ops(dstq, buf,
                                       slice(s * 512, (s + 1) * 512), pk[:])
                        yield
                wv1 = w_sb[WC_V1]
                for kbq in range(4):
                    pv1 = psBG.tile([128, 512], F32, tag="bgk",
                                    name=f"bgv{kbq}")
                    for kj in range(4):
                        kb = 4 * kbq + kj
                        for kt in range(NK):
                            nc.tensor.matmul(
                                pv1[:, kj * 128:(kj + 1) * 128],
                                x_sb[kt][:, kb * 128:(kb + 1) * 128],
                                wv1[:, kt * 128:(kt + 1) * 128],
                                start=(kt == 0), stop=(kt == NK - 1),
                                skip_group_check=True)
                        yield
                    for kj in range(4):
                        kb = 4 * kbq + kj
                        nc.gpsimd.tensor_copy(
                            vap3[1][:, :, kb, 0:64],
                            pv1[:, kj * 128:(kj + 1) * 128]
                            .rearrange("a (h c) -> a h c", h=2))
                    yield

            def bg_oproj(si, pool):
                t0 = si * 512
                for n in range(8):
                    pD = pool.tile([128, 512], F32, tag="pd",
                                   name=f"pD{si}_{n}")
                    for p in range(2):
                        nc.tensor.matmul(
                            pD[:], wo_sb[p][:, n * 128:(n + 1) * 128],
                            oT[p][:, t0:t0 + 512],
                            start=(p == 0), stop=(p == 1))
                    fo = fop.tile([128, 512], BF16, tag="fo")
                    if n % 2 == 0:
                        nc.vector.tensor_copy(fo[:], pD[:])
                    else:
                        nc.gpsimd.tensor_copy(fo[:], pD[:])
                    nc.sync.dma_start(
                        out=d_out[n * 128:(n + 1) * 128, t0:t0 + 512],
                        in_=fo[:])
                    yield

            def drain(gen, count=10 ** 9):
                if gen is None:
                    return
                for _ in range(count):
                    try:
                        next(gen)
                    except StopIteration:
                        return

            def attn_strip(si, p, bg=None, bg_per_kb=2):
                """Attention for q-strip si, pair p; AV lags one block."""
                q0 = 512 * si
                kb_max = 4 * (si + 1)
                av = [psA.tile([65, 512], F32, tag=f"av{hl}",
                               name=f"av{si}_{p}_{hl}") for hl in range(2)]

                def emit_av(st):
                    kb, ptb, o, L = st
                    for hl in range(2):
                        nc.tensor.matmul(
                            av[hl][:, o:512],
                            vap3[p][:, hl, kb, :],
                            ptb[:, 512 * hl:512 * hl + L],
                            start=(kb == 0), stop=(kb == kb_max - 1),
                            skip_group_check=True)

                prev = None
                for kb in range(kb_max):
                    o = max(0, 128 * kb - q0)
                    L = 512 - o
                    sps = psS.tile([128, 1024], F32, tag="sps",
                                   name=f"sps{si}_{p}_{kb}")
                    for hl in range(2):
                        hb = 64 * hl
                        nc.tensor.matmul(
                            sps[:, 512 * hl + o:512 * hl + 512],
                            kT[p][hb:hb + 64, kb * 128:(kb + 1) * 128],
                            qT[p][hb:hb + 64, q0 + o:q0 + 512],
                            start=True, stop=True)
                    ptb = ptbp.tile([128, 1024], BF16, tag="ptb",
                                    name=f"ptb{si}_{p}_{kb}")
                    sps3 = sps[:].rearrange("a (h q) -> a h q", h=2)
                    ptb3 = ptb[:].rearrange("a (h q) -> a h q", h=2)
                    nc.scalar.activation(ptb3[:, :, 0:L], sps3[:, :, o:512],
                                         AF.Exp, scale=SCALE)
                    if kb >= 4 * si:
                        nc.vector.tensor_tensor(
                            out=ptb3[:, :, 0:128], in0=ptb3[:, :, 0:128],
                            in1=tri3, op=ALU.mult)
                    if prev is not None:
                        emit_av(prev)
                    prev = (kb, ptb, o, L)
                    if bg is not None:
                        drain(bg, count=bg_per_kb)
                emit_av(prev)
                # normalize -> oT strip
                for hl in range(2):
                    r_sb = rp.tile([1, 512], F32, tag=f"r{hl}",
                                   name=f"rsb{si}_{p}_{hl}")
                    nc.vector.reciprocal(r_sb[:], av[hl][64:65, :])
                    rb = rp.tile([64, 512], F32, tag=f"rb{hl}",
                                 name=f"rbb{si}_{p}_{hl}")
                    nc.gpsimd.partition_broadcast(rb[:], r_sb[:])
                    nc.vector.tensor_tensor(
                        out=oT[p][64 * hl:64 * hl + 64, q0:q0 + 512],
                        in0=av[hl][0:64, :], in1=rb[:], op=ALU.mult)

            # ---------------- Phase B: attention --------------------------
            bg1 = bg_pair1()
            for si in (3, 2, 1, 0):
                attn_strip(si, 0, bg=bg1, bg_per_kb=1)

            # leftover pair-1 work fills the first p1 strip (its out-proj
            # is not available yet)
            attn_strip(3, 1, bg=bg1, bg_per_kb=2)
            drain(bg1)
            obg = bg_oproj(3, psD)
            for si in (2, 1, 0):
                attn_strip(si, 1, bg=obg, bg_per_kb=2)
                drain(obg)
                obg = bg_oproj(si, psD) if si > 0 else None

            psD.release()
            psBG.release()
            psA.release()
            psS.release()

            # tail: out-projection of the last (smallest) strip with all
            # banks available
            psD2 = tc.alloc_tile_pool(name="psD2", bufs=4, space="PSUM")
            drain(bg_oproj(0, psD2))
            psD2.release()

            if dbg:
                nc.sync.dma_start(out=d_dbg_q0[:], in_=qT[0][:])
                nc.sync.dma_start(out=d_dbg_k0[:], in_=kT[0][:])
                nc.sync.dma_start(out=d_dbg_va0[:], in_=vap[0][:])
                nc.sync.dma_start(out=d_dbg_o0[:], in_=oT[0][:])

    nc.compile()
    return nc


_NC_CACHE = None


def _get_program():
    global _NC_CACHE
    if _NC_CACHE is None:
        _NC_CACHE = _build_program()
    return _NC_CACHE


def _rope_tables():
    inv_freq = 1.0 / (10000.0 ** (np.arange(0, HD, 2, dtype=np.float32) / HD))
    freqs = np.outer(np.arange(T, dtype=np.float32), inv_freq)  # [T, 32]
    emb = np.concatenate([freqs, freqs], axis=-1)               # [T, 64]
    return np.cos(emb), np.sin(emb)


def _to_bf16(a):
    import ml_dtypes
    return np.asarray(a, dtype=np.float32).astype(ml_dtypes.bfloat16)


def _swizzle_w(wcol):
    """[D, 128] column block -> SBUF image [128, NK*128] with
    img[p, kt*128 + j] = wcol[kt*128 + p, j]."""
    w3 = wcol.reshape(NK, 128, 128)          # [kt, p, j]
    return np.ascontiguousarray(w3.transpose(1, 0, 2).reshape(128, NK * 128))


def _host_prep(x, w_qkv, w_out):
    cos, sin = _rope_tables()
    cosP = np.ascontiguousarray(cos.T[PI, :])                   # [64, T]
    sinP = sin.T[PI, :].copy()
    sinP[0::2, :] *= -1.0                                       # sign baked in
    cos2 = _to_bf16(np.vstack([cosP, cosP]))
    sin2 = _to_bf16(np.vstack([sinP, sinP]))
    tri = np.triu(np.ones((128, 128), dtype=np.float32))        # keep j >= i
    trimask = _to_bf16(np.ascontiguousarray(
        np.concatenate([tri, tri], axis=1)))                    # [128, 256]

    in_maps = []
    for core in range(NCORES):
        b = core // GROUPS
        h0 = (core % GROUPS) * HPC
        xT = _to_bf16(np.ascontiguousarray(x[b].T))             # [D, T]

        def wcolq(kind, pair):                                  # permuted
            cols = []
            for hh in range(2):
                h = h0 + 2 * pair + hh
                wcol = w_qkv[:, kind * D + h * HD:kind * D + (h + 1) * HD]
                cols.append(wcol[:, PI])
            return np.concatenate(cols, axis=1)                 # [D, 128]

        def wcolv(pair):
            cols = []
            for hh in range(2):
                h = h0 + 2 * pair + hh
                cols.append(w_qkv[:, 2 * D + h * HD:2 * D + (h + 1) * HD])
            return np.concatenate(cols, axis=1)

        order = [wcolq(0, 0), wcolq(1, 0), wcolv(0), wcolv(1),
                 wcolq(0, 1), wcolq(1, 1)]
        wimg = np.concatenate([_swizzle_w(c) for c in order], axis=1)
        wimg = np.ascontiguousarray(_to_bf16(wimg))             # [128, 6144]

        wo_rows = w_out[h0 * HD:(h0 + HPC) * HD, :]             # [256, D]
        woimg = np.ascontiguousarray(_to_bf16(
            np.concatenate([wo_rows[0:128, :], wo_rows[128:256, :]],
                           axis=1)))                            # [128, 2D]

        in_maps.append({
            "xT": xT,
            "wimg": wimg,
            "woimg": woimg,
            "cos2": cos2,
            "sin2": sin2,
            "trimask": trimask,
        })
    return in_maps


def kernel(x, w_qkv, w_out):
    x = np.asarray(x, dtype=np.float32)
    w_qkv = np.asarray(w_qkv, dtype=np.float32)
    w_out = np.asarray(w_out, dtype=np.float32)
    nc = _get_program()
    in_maps = _host_prep(x, w_qkv, w_out)
    trace = bool(int(os.environ.get("KBENCH_TRACE", "0")))
    res = run_bass_kernel_spmd(nc, in_maps, list(range(NCORES)), trace=trace)
    if trace and res.exec_time_ns is not None:
        print(f"HW exec time: {res.exec_time_ns} ns")
    out = np.zeros((B, T, D), dtype=np.float32)
    for core in range(NCORES):
        b = core // GROUPS
        out[b] += res.results[core]["outp"].T.astype(np.float32)
    return out
